# revision 23
# baseline (speedup 1.0000x reference)
"""Causal self-attention (strictly-causal, masked-center) Trainium2 kernel.

Full inputs in, full outputs out. Data-parallel over batch across 8 cores
(4 batches per core). Everything on-device is bf16 with fp32 PSUM
accumulation; attention is computed in transposed (k-major) layout so no
on-device transposes are needed:

  qT,kT : (channels, S) channel-major from 1x1-conv projections
  vT    : (S, channels) computed directly transposed by using x as the
          stationary matmul operand
  scoresT[k,q] = sum_d kT[d,k] qT[d,q]      (dk=32 contraction, 4 heads
          packed into the 128x128 PE array via tile_position row-tiling)
  exp on ScalarE straight out of PSUM (no max subtraction: |scores|<~9)
  attn@V and the softmax denominator are col-tiled matmuls over the same
          exp tiles; division happens after, via a tiny select-matrix
          broadcast matmul + one tensor_tensor multiply per chunk.
"""
import numpy as np

import concourse.bass as bass
import concourse.bacc as bacc
import concourse.mybir as mybir
from concourse.tile import TileContext
from concourse.bass_utils import run_bass_kernel_spmd

N_CORES = 8
B, C, H, W = 32, 256, 32, 32
S = H * W
BPC = B // N_CORES
NH, DK = 8, 32
FP32 = mybir.dt.float32
BF16 = mybir.dt.bfloat16
BF16_NP = mybir.dt.np(BF16)

_cache = {}


def _build():
    nc = bacc.Bacc("TRN2", target_bir_lowering=False, debug=False,
                   num_devices=N_CORES)
    xs = nc.dram_tensor("xs", [BPC, C, S], FP32, kind="ExternalInput")
    qk_wT = nc.dram_tensor("qk_wT", [C, 512], BF16, kind="ExternalInput")
    qk_b = nc.dram_tensor("qk_b", [128, 4], FP32, kind="ExternalInput")
    v_wT = nc.dram_tensor("v_wT", [C, 512], BF16, kind="ExternalInput")
    v_b = nc.dram_tensor("v_b", [1, 512], BF16, kind="ExternalInput")
    maskT = nc.dram_tensor("maskT", [128, 128], BF16, kind="ExternalInput")
    sel = nc.dram_tensor("sel", [64, 128], FP32, kind="ExternalInput")
    out = nc.dram_tensor("out", [BPC, C, S], FP32, kind="ExternalOutput")

    EXP = mybir.ActivationFunctionType.Exp

    with TileContext(nc) as tc:
        with (
            tc.tile_pool(name="const", bufs=1) as cpool,
            tc.tile_pool(name="data", bufs=2) as dpool,
            tc.tile_pool(name="expp", bufs=6) as epool,
            tc.tile_pool(name="small", bufs=4) as spool,
            tc.tile_pool(name="ps", bufs=4, space="PSUM") as pspool,
            tc.tile_pool(name="sc", bufs=2, space="PSUM") as scpool,
        ):
            # ---------------- constants ----------------
            qkw_sb = []
            for ci in range(2):
                t = cpool.tile([128, 512], BF16, tag=f"qkw{ci}")
                nc.sync.dma_start(out=t[:], in_=qk_wT[128 * ci:128 * ci + 128, :])
                qkw_sb.append(t)
            vw_sb = []
            for ci in range(2):
                t = cpool.tile([128, 512], BF16, tag=f"vw{ci}")
                nc.sync.dma_start(out=t[:], in_=v_wT[128 * ci:128 * ci + 128, :])
                vw_sb.append(t)
            qkb_sb = cpool.tile([128, 4], FP32, tag="qkb")
            nc.sync.dma_start(out=qkb_sb[:], in_=qk_b[:])
            vb_sb = cpool.tile([1, 512], BF16, tag="vb")
            nc.sync.dma_start(out=vb_sb[:], in_=v_b[:])
            maskT_sb = cpool.tile([128, 128], BF16, tag="mask")
            nc.sync.dma_start(out=maskT_sb[:], in_=maskT[:])
            sel_sb = cpool.tile([64, 128], FP32, tag="sel")
            nc.sync.dma_start(out=sel_sb[:], in_=sel[:])
            ones_row = cpool.tile([1, 512], BF16, tag="ones_row")
            nc.vector.memset(ones_row[:], 1.0)
            eps_row = cpool.tile([1, 32], BF16, tag="eps_row")
            nc.vector.memset(eps_row[:], 1e-20)

            for b in range(BPC):
                # ---------------- load + cast x ----------------
                xfb = []
                for ci in range(2):
                    t32 = dpool.tile([128, S], FP32, tag=f"xf32_{ci}")
                    nc.sync.dma_start(out=t32[:],
                                      in_=xs[b, 128 * ci:128 * ci + 128, :])
                    tb = dpool.tile([128, S], BF16, tag=f"xf_{ci}")
                    nc.vector.tensor_copy(tb[:], t32[:])
                    xfb.append(tb)

                # ---------------- q/k projections (channel-major) -------
                q_sb = [dpool.tile([128, S], BF16, tag=f"q{m}", name=f"q{m}") for m in range(2)]
                k_sb = [dpool.tile([128, S], BF16, tag=f"k{m}", name=f"k{m}") for m in range(2)]
                for bi, (off, dest) in enumerate(((0, q_sb), (256, k_sb))):
                    for m in range(2):
                        for n_ in range(2):
                            ps = pspool.tile([128, 512], FP32, tag="ps")
                            for kc in range(2):
                                nc.tensor.matmul(
                                    ps[:],
                                    lhsT=qkw_sb[kc][:, off + 128 * m:off + 128 * m + 128],
                                    rhs=xfb[kc][:, 512 * n_:512 * n_ + 512],
                                    start=(kc == 0), stop=(kc == 1))
                            nc.vector.tensor_scalar_add(
                                dest[m][:, 512 * n_:512 * n_ + 512], ps[:],
                                qkb_sb[:, 2 * bi + m:2 * bi + m + 1])

                # ---------------- vT projection (S-major) ---------------
                vT_sb = []
                for j in range(8):
                    ps = pspool.tile([128, 512], FP32, tag="ps")
                    for kc in range(2):
                        nc.tensor.matmul(
                            ps[:],
                            lhsT=xfb[kc][:, 128 * j:128 * j + 128],
                            rhs=vw_sb[kc][:],
                            start=(kc == 0), stop=False)
                    nc.tensor.matmul(ps[:], lhsT=ones_row[0:1, 0:128],
                                     rhs=vb_sb[:], start=False, stop=True)
                    vt = dpool.tile([128, 512], BF16, tag=f"vT{j}")
                    nc.vector.tensor_copy(vt[:], ps[:])
                    vT_sb.append(vt)

                # ---------------- attention ----------------
                out_sb = [dpool.tile([128, S], FP32, tag=f"o{ci}", name=f"o{ci}")
                          for ci in range(2)]
                for half in range(2):
                    qlo = 512 * half
                    av_ps = [pspool.tile([128, 512], FP32, tag="ps", name="avps")
                             for _ in range(4)]
                    last_kj = 4 * half + 3
                    for kj in range(last_kj + 1):
                        d = max(0, 128 * kj - qlo)  # cols left of diagonal
                        width = 512 - d
                        for g in range(2):
                            exs = []
                            for t in range(2):
                                # 2 heads per (128,1024) scores tile: 2 PSUM
                                # banks, so two tiles double-buffer in 4 banks
                                # and TensorE overlaps ScalarE's exp.
                                scp = scpool.tile([128, 1024], FP32, tag="sc",
                                                  name="scp")
                                for j in range(2):
                                    i = 2 * t + j
                                    nc.tensor.matmul(
                                        scp[:, 512 * j + d:512 * j + 512],
                                        lhsT=k_sb[g][32 * i:32 * i + 32,
                                                     128 * kj:128 * kj + 128],
                                        rhs=q_sb[g][32 * i:32 * i + 32,
                                                    qlo + d:qlo + 512],
                                        start=True, stop=True,
                                        tile_position=(32 * i, 0))
                                ex = epool.tile([128, 1024], BF16, tag="exp",
                                                name="ex")
                                sc3 = scp[:].rearrange("p (h c) -> p h c", h=2)
                                ex3 = ex[:].rearrange("p (h c) -> p h c", h=2)
                                nc.scalar.activation(ex3[:, :, d:512],
                                                     sc3[:, :, d:512], EXP)
                                if kj >= 4 * half:
                                    # strict causal mask on the diagonal block
                                    msl = ex3[:, :, d:d + 128]
                                    nc.gpsimd.tensor_mul(
                                        msl, msl,
                                        maskT_sb[:, None, :].broadcast_to(
                                            (128, 2, 128)))
                                exs.append(ex)
                            for i in range(4):
                                h = 4 * g + i
                                t, j = i // 2, i % 2
                                nc.tensor.matmul(
                                    av_ps[2 * g + t][64 * j:64 * j + 64, d:512],
                                    lhsT=vT_sb[kj][:, 64 * h:64 * h + 64],
                                    rhs=exs[t][:, 512 * j + d:512 * j + 512],
                                    start=(kj == 0), stop=(kj == last_kj),
                                    skip_group_check=True,
                                    tile_position=(0, 64 * j))
                                if kj == 0:
                                    # += 1e-20 on the den rows so 1/x is
                                    # defined even for fully-masked queries
                                    # (q=0): 0*1e20 = 0 matches reference.
                                    nc.tensor.matmul(
                                        av_ps[2 * g + t][64 * j + 32:
                                                         64 * j + 64, 0:512],
                                        lhsT=eps_row[:],
                                        rhs=ones_row[:],
                                        start=False, stop=False,
                                        skip_group_check=True,
                                        tile_position=(0, 64 * j + 32))

                    # ------- softmax division + output assembly -------
                    # each av pair tile: head A rows 0-31 (+den at 32), head
                    # B rows 64-95 (+den at 96); den rows are >= 1e-20 thanks
                    # to the eps matmul, so reciprocal directly on PSUM is
                    # safe. The select matmul broadcasts each den row over
                    # its 32 output channels.
                    for g in range(2):
                        bct = scpool.tile([128, 1024], FP32, tag="sc",
                                          name="bct")
                        bc_sb = spool.tile([128, 1024], FP32, tag="bcs",
                                           name=f"bcs{g}")
                        for t in range(2):
                            p = 2 * g + t
                            den_sb = spool.tile([64, 512], FP32, tag="den",
                                                name=f"den{p}")
                            # reciprocal_approx_fast reads garbage from PSUM
                            # on hardware; stage the den rows in SBUF first.
                            nc.vector.tensor_copy(den_sb[0:32, :],
                                                  av_ps[p][32:64, :])
                            nc.vector.tensor_copy(den_sb[32:64, :],
                                                  av_ps[p][96:128, :])
                            recip_f = spool.tile([64, 512], FP32, tag="recf",
                                                 name=f"recf{p}")
                            nc.vector.reciprocal_approx_fast(recip_f[:],
                                                             den_sb[:])
                            nc.tensor.matmul(
                                bct[:, 512 * t:512 * t + 512],
                                lhsT=sel_sb[:], rhs=recip_f[:],
                                start=True, stop=True)
                        nc.vector.tensor_copy(bc_sb[:], bct[:])
                        for t in range(2):
                            p = 2 * g + t
                            nc.vector.tensor_mul(
                                out_sb[g][64 * t:64 * t + 32, qlo:qlo + 512],
                                av_ps[p][0:32, :], bc_sb[0:32, 512 * t:512 * t + 512])
                            nc.vector.tensor_mul(
                                out_sb[g][64 * t + 32:64 * t + 64,
                                          qlo:qlo + 512],
                                av_ps[p][64:96, :],
                                bc_sb[64:96, 512 * t:512 * t + 512])

                for ci in range(2):
                    nc.sync.dma_start(out=out[b, 128 * ci:128 * ci + 128, :],
                                      in_=out_sb[ci][:])

    nc.compile()
    return nc


def _host_consts(q_w, q_b, kv_w, kv_b):
    scale = 1.0 / np.sqrt(DK)
    qk_wT = np.concatenate([q_w.T * scale, kv_w[:256].T], axis=1)
    qk_b_ = np.stack([q_b[:128] * scale, q_b[128:] * scale,
                      kv_b[:128], kv_b[128:256]], axis=1).astype(np.float32)
    # per-head 64-col layout: 32 v cols | ones col (denominator) | 31 zeros
    v_wT_aug = np.zeros((256, 512), np.float32)
    v_b_aug = np.zeros((1, 512), np.float32)
    vw = kv_w[256:].T
    for h in range(NH):
        v_wT_aug[:, 64 * h:64 * h + 32] = vw[:, 32 * h:32 * h + 32]
        v_b_aug[0, 64 * h:64 * h + 32] = kv_b[256 + 32 * h:256 + 32 * h + 32]
        v_b_aug[0, 64 * h + 32] = 1.0
    kidx = np.arange(128)
    maskT = (kidx[:, None] < kidx[None, :]).astype(np.float32)
    ridx = np.arange(128)
    kk = np.arange(64)
    sel_m = ((ridx[None, :] < 64) & (kk[:, None] == 0) |
             (ridx[None, :] >= 64) & (kk[:, None] == 32)).astype(np.float32)
    return {
        "sel": sel_m,
        "qk_wT": qk_wT.astype(BF16_NP),
        "qk_b": qk_b_,
        "v_wT": v_wT_aug.astype(BF16_NP),
        "v_b": v_b_aug.astype(BF16_NP),
        "maskT": maskT.astype(BF16_NP),
    }


def get_program():
    if "nc" not in _cache:
        _cache["nc"] = _build()
    return _cache["nc"]


def run(x, q_w, q_b, kv_w, kv_b, trace=False):
    nc = get_program()
    consts = _host_consts(np.asarray(q_w, np.float32), np.asarray(q_b, np.float32),
                          np.asarray(kv_w, np.float32), np.asarray(kv_b, np.float32))
    x = np.asarray(x, np.float32)
    in_maps = []
    for ci in range(N_CORES):
        m = dict(consts)
        m["xs"] = np.ascontiguousarray(
            x[ci * BPC:(ci + 1) * BPC].reshape(BPC, C, S))
        in_maps.append(m)
    res = run_bass_kernel_spmd(nc, in_maps, list(range(N_CORES)), trace=trace)
    outs = [res.results[ci]["out"] for ci in range(N_CORES)]
    full = np.concatenate(outs, axis=0).reshape(B, C, H, W).astype(np.float32)
    return full, res


def kernel(x, q_w, q_b, kv_w, kv_b):
    full, _ = run(x, q_w, q_b, kv_w, kv_b, trace=False)
    return full


# revision 24
# speedup vs baseline: 1.0440x; 1.0440x over previous
"""Causal self-attention (strictly-causal, masked-center) Trainium2 kernel.

Full inputs in, full outputs out. Data-parallel over batch across 8 cores
(4 batches per core). Everything on-device is bf16 with fp32 PSUM
accumulation; attention is computed in transposed (k-major) layout so no
on-device transposes are needed:

  qT,kT : (channels, S) channel-major from 1x1-conv projections
  vT    : (S, channels) computed directly transposed by using x as the
          stationary matmul operand
  scoresT[k,q] = sum_d kT[d,k] qT[d,q]      (dk=32 contraction, 4 heads
          packed into the 128x128 PE array via tile_position row-tiling)
  exp on ScalarE straight out of PSUM (no max subtraction: |scores|<~9)
  attn@V and the softmax denominator are col-tiled matmuls over the same
          exp tiles; division happens after, via a tiny select-matrix
          broadcast matmul + one tensor_tensor multiply per chunk.
"""
import numpy as np

import concourse.bass as bass
import concourse.bacc as bacc
import concourse.mybir as mybir
from concourse.tile import TileContext
from concourse.bass_utils import run_bass_kernel_spmd

N_CORES = 8
B, C, H, W = 32, 256, 32, 32
S = H * W
BPC = B // N_CORES
NH, DK = 8, 32
FP32 = mybir.dt.float32
BF16 = mybir.dt.bfloat16
BF16_NP = mybir.dt.np(BF16)

_cache = {}


def _build():
    nc = bacc.Bacc("TRN2", target_bir_lowering=False, debug=False,
                   num_devices=N_CORES)
    xs = nc.dram_tensor("xs", [BPC, C, S], FP32, kind="ExternalInput")
    qk_wT = nc.dram_tensor("qk_wT", [C, 512], BF16, kind="ExternalInput")
    qk_b = nc.dram_tensor("qk_b", [128, 4], FP32, kind="ExternalInput")
    v_wT = nc.dram_tensor("v_wT", [C, 512], BF16, kind="ExternalInput")
    v_b = nc.dram_tensor("v_b", [1, 512], BF16, kind="ExternalInput")
    maskT = nc.dram_tensor("maskT", [128, 128], BF16, kind="ExternalInput")
    sel = nc.dram_tensor("sel", [64, 128], FP32, kind="ExternalInput")
    out = nc.dram_tensor("out", [BPC, C, S], FP32, kind="ExternalOutput")

    EXP = mybir.ActivationFunctionType.Exp

    with TileContext(nc) as tc:
        with (
            tc.tile_pool(name="const", bufs=1) as cpool,
            tc.tile_pool(name="data", bufs=2) as dpool,
            tc.tile_pool(name="expp", bufs=10) as epool,
            tc.tile_pool(name="small", bufs=4) as spool,
            tc.tile_pool(name="ps", bufs=4, space="PSUM") as pspool,
            tc.tile_pool(name="sc", bufs=2, space="PSUM") as scpool,
        ):
            # ---------------- constants ----------------
            qkw_sb = []
            for ci in range(2):
                t = cpool.tile([128, 512], BF16, tag=f"qkw{ci}")
                nc.sync.dma_start(out=t[:], in_=qk_wT[128 * ci:128 * ci + 128, :])
                qkw_sb.append(t)
            vw_sb = []
            for ci in range(2):
                t = cpool.tile([128, 512], BF16, tag=f"vw{ci}")
                nc.sync.dma_start(out=t[:], in_=v_wT[128 * ci:128 * ci + 128, :])
                vw_sb.append(t)
            qkb_sb = cpool.tile([128, 4], FP32, tag="qkb")
            nc.sync.dma_start(out=qkb_sb[:], in_=qk_b[:])
            vb_sb = cpool.tile([1, 512], BF16, tag="vb")
            nc.sync.dma_start(out=vb_sb[:], in_=v_b[:])
            maskT_sb = cpool.tile([128, 128], BF16, tag="mask")
            nc.sync.dma_start(out=maskT_sb[:], in_=maskT[:])
            sel_sb = cpool.tile([64, 128], FP32, tag="sel")
            nc.sync.dma_start(out=sel_sb[:], in_=sel[:])
            ones_row = cpool.tile([1, 512], BF16, tag="ones_row")
            nc.vector.memset(ones_row[:], 1.0)
            eps_row = cpool.tile([1, 32], BF16, tag="eps_row")
            nc.vector.memset(eps_row[:], 1e-20)

            for b in range(BPC):
                # ---------------- load + cast x ----------------
                xfb = []
                for ci in range(2):
                    t32 = dpool.tile([128, S], FP32, tag=f"xf32_{ci}")
                    nc.sync.dma_start(out=t32[:],
                                      in_=xs[b, 128 * ci:128 * ci + 128, :])
                    tb = dpool.tile([128, S], BF16, tag=f"xf_{ci}")
                    nc.vector.tensor_copy(tb[:], t32[:])
                    xfb.append(tb)

                # ---------------- q/k projections (channel-major) -------
                q_sb = [dpool.tile([128, S], BF16, tag=f"q{m}", name=f"q{m}") for m in range(2)]
                k_sb = [dpool.tile([128, S], BF16, tag=f"k{m}", name=f"k{m}") for m in range(2)]
                for bi, (off, dest) in enumerate(((0, q_sb), (256, k_sb))):
                    for m in range(2):
                        for n_ in range(2):
                            ps = pspool.tile([128, 512], FP32, tag="ps")
                            for kc in range(2):
                                nc.tensor.matmul(
                                    ps[:],
                                    lhsT=qkw_sb[kc][:, off + 128 * m:off + 128 * m + 128],
                                    rhs=xfb[kc][:, 512 * n_:512 * n_ + 512],
                                    start=(kc == 0), stop=(kc == 1))
                            nc.vector.tensor_scalar_add(
                                dest[m][:, 512 * n_:512 * n_ + 512], ps[:],
                                qkb_sb[:, 2 * bi + m:2 * bi + m + 1])

                # ---------------- vT projection (S-major) ---------------
                vT_sb = []
                for j in range(8):
                    ps = pspool.tile([128, 512], FP32, tag="ps")
                    for kc in range(2):
                        nc.tensor.matmul(
                            ps[:],
                            lhsT=xfb[kc][:, 128 * j:128 * j + 128],
                            rhs=vw_sb[kc][:],
                            start=(kc == 0), stop=False)
                    nc.tensor.matmul(ps[:], lhsT=ones_row[0:1, 0:128],
                                     rhs=vb_sb[:], start=False, stop=True)
                    vt = dpool.tile([128, 512], BF16, tag=f"vT{j}")
                    nc.vector.tensor_copy(vt[:], ps[:])
                    vT_sb.append(vt)

                # ---------------- attention ----------------
                out_sb = [dpool.tile([128, S], FP32, tag=f"o{ci}", name=f"o{ci}")
                          for ci in range(2)]
                for half in range(2):
                    qlo = 512 * half
                    av_ps = [pspool.tile([128, 512], FP32, tag="ps", name="avps")
                             for _ in range(4)]
                    last_kj = 4 * half + 3
                    for kj in range(last_kj + 1):
                        d = max(0, 128 * kj - qlo)  # cols left of diagonal
                        width = 512 - d
                        for g in range(2):
                            exs = []
                            for t in range(2):
                                # 2 heads per (128,1024) scores tile: 2 PSUM
                                # banks, so two tiles double-buffer in 4 banks
                                # and TensorE overlaps ScalarE's exp.
                                scp = scpool.tile([128, 1024], FP32, tag="sc",
                                                  name="scp")
                                for j in range(2):
                                    i = 2 * t + j
                                    nc.tensor.matmul(
                                        scp[:, 512 * j + d:512 * j + 512],
                                        lhsT=k_sb[g][32 * i:32 * i + 32,
                                                     128 * kj:128 * kj + 128],
                                        rhs=q_sb[g][32 * i:32 * i + 32,
                                                    qlo + d:qlo + 512],
                                        start=True, stop=True,
                                        tile_position=(32 * i, 0))
                                ex = epool.tile([128, 1024], BF16, tag="exp",
                                                name="ex")
                                sc3 = scp[:].rearrange("p (h c) -> p h c", h=2)
                                ex3 = ex[:].rearrange("p (h c) -> p h c", h=2)
                                nc.scalar.activation(ex3[:, :, d:512],
                                                     sc3[:, :, d:512], EXP)
                                if kj >= 4 * half:
                                    # strict causal mask on the diagonal block
                                    msl = ex3[:, :, d:d + 128]
                                    nc.vector.tensor_mul(
                                        msl, msl,
                                        maskT_sb[:, None, :].broadcast_to(
                                            (128, 2, 128)))
                                exs.append(ex)
                            for i in range(4):
                                h = 4 * g + i
                                t, j = i // 2, i % 2
                                nc.tensor.matmul(
                                    av_ps[2 * g + t][64 * j:64 * j + 64, d:512],
                                    lhsT=vT_sb[kj][:, 64 * h:64 * h + 64],
                                    rhs=exs[t][:, 512 * j + d:512 * j + 512],
                                    start=(kj == 0), stop=(kj == last_kj),
                                    skip_group_check=True,
                                    tile_position=(0, 64 * j))
                                if kj == 0:
                                    # += 1e-20 on the den rows so 1/x is
                                    # defined even for fully-masked queries
                                    # (q=0): 0*1e20 = 0 matches reference.
                                    nc.tensor.matmul(
                                        av_ps[2 * g + t][64 * j + 32:
                                                         64 * j + 64, 0:512],
                                        lhsT=eps_row[:],
                                        rhs=ones_row[:],
                                        start=False, stop=False,
                                        skip_group_check=True,
                                        tile_position=(0, 64 * j + 32))

                    # ------- softmax division + output assembly -------
                    # each av pair tile: head A rows 0-31 (+den at 32), head
                    # B rows 64-95 (+den at 96); den rows are >= 1e-20 thanks
                    # to the eps matmul, so reciprocal directly on PSUM is
                    # safe. The select matmul broadcasts each den row over
                    # its 32 output channels.
                    for g in range(2):
                        bct = scpool.tile([128, 1024], FP32, tag="sc",
                                          name="bct")
                        bc_sb = spool.tile([128, 1024], FP32, tag="bcs",
                                           name=f"bcs{g}")
                        for t in range(2):
                            p = 2 * g + t
                            den_sb = spool.tile([64, 512], FP32, tag="den",
                                                name=f"den{p}")
                            # reciprocal_approx_fast reads garbage from PSUM
                            # on hardware; stage the den rows in SBUF first.
                            nc.vector.tensor_copy(den_sb[0:32, :],
                                                  av_ps[p][32:64, :])
                            nc.vector.tensor_copy(den_sb[32:64, :],
                                                  av_ps[p][96:128, :])
                            recip_f = spool.tile([64, 512], FP32, tag="recf",
                                                 name=f"recf{p}")
                            nc.vector.reciprocal_approx_fast(recip_f[:],
                                                             den_sb[:])
                            nc.tensor.matmul(
                                bct[:, 512 * t:512 * t + 512],
                                lhsT=sel_sb[:], rhs=recip_f[:],
                                start=True, stop=True)
                        nc.vector.tensor_copy(bc_sb[:], bct[:])
                        for t in range(2):
                            p = 2 * g + t
                            nc.vector.tensor_mul(
                                out_sb[g][64 * t:64 * t + 32, qlo:qlo + 512],
                                av_ps[p][0:32, :], bc_sb[0:32, 512 * t:512 * t + 512])
                            nc.vector.tensor_mul(
                                out_sb[g][64 * t + 32:64 * t + 64,
                                          qlo:qlo + 512],
                                av_ps[p][64:96, :],
                                bc_sb[64:96, 512 * t:512 * t + 512])

                for ci in range(2):
                    nc.sync.dma_start(out=out[b, 128 * ci:128 * ci + 128, :],
                                      in_=out_sb[ci][:])

    nc.compile()
    return nc


def _host_consts(q_w, q_b, kv_w, kv_b):
    scale = 1.0 / np.sqrt(DK)
    qk_wT = np.concatenate([q_w.T * scale, kv_w[:256].T], axis=1)
    qk_b_ = np.stack([q_b[:128] * scale, q_b[128:] * scale,
                      kv_b[:128], kv_b[128:256]], axis=1).astype(np.float32)
    # per-head 64-col layout: 32 v cols | ones col (denominator) | 31 zeros
    v_wT_aug = np.zeros((256, 512), np.float32)
    v_b_aug = np.zeros((1, 512), np.float32)
    vw = kv_w[256:].T
    for h in range(NH):
        v_wT_aug[:, 64 * h:64 * h + 32] = vw[:, 32 * h:32 * h + 32]
        v_b_aug[0, 64 * h:64 * h + 32] = kv_b[256 + 32 * h:256 + 32 * h + 32]
        v_b_aug[0, 64 * h + 32] = 1.0
    kidx = np.arange(128)
    maskT = (kidx[:, None] < kidx[None, :]).astype(np.float32)
    ridx = np.arange(128)
    kk = np.arange(64)
    sel_m = ((ridx[None, :] < 64) & (kk[:, None] == 0) |
             (ridx[None, :] >= 64) & (kk[:, None] == 32)).astype(np.float32)
    return {
        "sel": sel_m,
        "qk_wT": qk_wT.astype(BF16_NP),
        "qk_b": qk_b_,
        "v_wT": v_wT_aug.astype(BF16_NP),
        "v_b": v_b_aug.astype(BF16_NP),
        "maskT": maskT.astype(BF16_NP),
    }


def get_program():
    if "nc" not in _cache:
        _cache["nc"] = _build()
    return _cache["nc"]


def run(x, q_w, q_b, kv_w, kv_b, trace=False):
    nc = get_program()
    consts = _host_consts(np.asarray(q_w, np.float32), np.asarray(q_b, np.float32),
                          np.asarray(kv_w, np.float32), np.asarray(kv_b, np.float32))
    x = np.asarray(x, np.float32)
    in_maps = []
    for ci in range(N_CORES):
        m = dict(consts)
        m["xs"] = np.ascontiguousarray(
            x[ci * BPC:(ci + 1) * BPC].reshape(BPC, C, S))
        in_maps.append(m)
    res = run_bass_kernel_spmd(nc, in_maps, list(range(N_CORES)), trace=trace)
    outs = [res.results[ci]["out"] for ci in range(N_CORES)]
    full = np.concatenate(outs, axis=0).reshape(B, C, H, W).astype(np.float32)
    return full, res


def kernel(x, q_w, q_b, kv_w, kv_b):
    full, _ = run(x, q_w, q_b, kv_w, kv_b, trace=False)
    return full


# revision 25
# speedup vs baseline: 1.4362x; 1.3756x over previous
"""Causal self-attention (strictly-causal, masked-center) Trainium2 kernel.

Full inputs in, full outputs out. Data-parallel over batch across 8 cores
(4 batches per core). Everything on-device is bf16 with fp32 PSUM
accumulation; attention is computed in transposed (k-major) layout so no
on-device transposes are needed:

  qT,kT : (channels, S) channel-major from 1x1-conv projections; the
          1/sqrt(dk) score scale and biases are folded in on the host /
          into the PSUM evacuation (per-partition tensor_scalar bias).
  vT    : (S, channels) computed directly transposed by using x as the
          stationary matmul operand; bias added via a host-broadcast tile.
  scoresT[k,q] = sum_d kT[d,k] qT[d,q]      (dk=32 contraction, 4 heads
          packed into the 128x128 PE array via tile_position row-tiling)
  exp on ScalarE straight out of PSUM (no max subtraction: |scores|<~9),
          causal block-skipping plus per-block column narrowing.
  attn@V and the softmax denominator are col-tiled matmuls over the same
          exp tiles; division happens afterwards via a select-matrix
          broadcast matmul + one tensor_tensor multiply per group.
"""
import numpy as np

import concourse.bass as bass
import concourse.bacc as bacc
import concourse.mybir as mybir
from concourse.tile import TileContext
from concourse.bass_utils import run_bass_kernel_spmd

N_CORES = 8
B, C, H, W = 32, 256, 32, 32
S = H * W
BPC = B // N_CORES
NH, DK = 8, 32
FP32 = mybir.dt.float32
BF16 = mybir.dt.bfloat16
BF16_NP = mybir.dt.np(BF16)

_cache = {}


def _build():
    nc = bacc.Bacc("TRN2", target_bir_lowering=False, debug=False,
                   num_devices=N_CORES)
    xs = nc.dram_tensor("xs", [BPC, C, S], FP32, kind="ExternalInput")
    qk_wT = nc.dram_tensor("qk_wT", [C, 512], BF16, kind="ExternalInput")
    qk_b = nc.dram_tensor("qk_b", [128, 4], FP32, kind="ExternalInput")
    v_wT = nc.dram_tensor("v_wT", [C, 256], BF16, kind="ExternalInput")
    v_b = nc.dram_tensor("v_b", [128, 256], BF16, kind="ExternalInput")
    maskT = nc.dram_tensor("maskT", [128, 128], BF16, kind="ExternalInput")
    sel = nc.dram_tensor("sel", [128, 128], FP32, kind="ExternalInput")
    out = nc.dram_tensor("out", [BPC, C, S], FP32, kind="ExternalOutput")

    EXP = mybir.ActivationFunctionType.Exp

    with TileContext(nc) as tc:
        with (
            tc.tile_pool(name="const", bufs=1) as cpool,
            tc.tile_pool(name="data", bufs=2) as dpool,
            tc.tile_pool(name="expp", bufs=10) as epool,
            tc.tile_pool(name="small", bufs=4) as spool,
            tc.tile_pool(name="ps", bufs=4, space="PSUM") as pspool,
            tc.tile_pool(name="sc", bufs=2, space="PSUM") as scpool,
        ):
            # ---------------- constants ----------------
            qkw_sb = []
            for ci in range(2):
                t = cpool.tile([128, 512], BF16, tag=f"qkw{ci}")
                nc.sync.dma_start(out=t[:], in_=qk_wT[128 * ci:128 * ci + 128, :])
                qkw_sb.append(t)
            vw_sb = []
            for ci in range(2):
                t = cpool.tile([128, 256], BF16, tag=f"vw{ci}")
                nc.sync.dma_start(out=t[:], in_=v_wT[128 * ci:128 * ci + 128, :])
                vw_sb.append(t)
            qkb_sb = cpool.tile([128, 4], FP32, tag="qkb")
            nc.sync.dma_start(out=qkb_sb[:], in_=qk_b[:])
            vb_sb = cpool.tile([128, 256], BF16, tag="vb")
            nc.sync.dma_start(out=vb_sb[:], in_=v_b[:])
            maskT_sb = cpool.tile([128, 128], BF16, tag="mask")
            nc.sync.dma_start(out=maskT_sb[:], in_=maskT[:])
            sel_sb = cpool.tile([128, 128], FP32, tag="sel")
            nc.sync.dma_start(out=sel_sb[:], in_=sel[:])
            ones_col = cpool.tile([128, 1], BF16, tag="ones_col")
            nc.vector.memset(ones_col[:], 1.0)

            for b in range(BPC):
                # ---------------- load + cast x ----------------
                xfb = []
                for ci in range(2):
                    t32 = dpool.tile([128, S], FP32, tag=f"xf32_{ci}")
                    nc.sync.dma_start(out=t32[:],
                                      in_=xs[b, 128 * ci:128 * ci + 128, :])
                    tb = dpool.tile([128, S], BF16, tag=f"xf_{ci}")
                    nc.vector.tensor_copy(tb[:], t32[:])
                    xfb.append(tb)

                # ---------------- q/k projections (channel-major) -------
                q_sb = [dpool.tile([128, S], BF16, tag=f"q{m}", name=f"q{m}")
                        for m in range(2)]
                k_sb = [dpool.tile([128, S], BF16, tag=f"k{m}", name=f"k{m}")
                        for m in range(2)]
                for bi, (off, dest) in enumerate(((0, q_sb), (256, k_sb))):
                    for m in range(2):
                        for n_ in range(2):
                            ps = pspool.tile([128, 512], FP32, tag="ps")
                            for kc in range(2):
                                nc.tensor.matmul(
                                    ps[:],
                                    lhsT=qkw_sb[kc][:, off + 128 * m:
                                                    off + 128 * m + 128],
                                    rhs=xfb[kc][:, 512 * n_:512 * n_ + 512],
                                    start=(kc == 0), stop=(kc == 1))
                            nc.vector.tensor_scalar_add(
                                dest[m][:, 512 * n_:512 * n_ + 512], ps[:],
                                qkb_sb[:, 2 * bi + m:2 * bi + m + 1])

                # ---------------- vT projection (S-major) ---------------
                vT_sb = []
                for j in range(8):
                    ps = pspool.tile([128, 256], FP32, tag="ps")
                    for kc in range(2):
                        nc.tensor.matmul(
                            ps[:],
                            lhsT=xfb[kc][:, 128 * j:128 * j + 128],
                            rhs=vw_sb[kc][:],
                            start=(kc == 0), stop=(kc == 1))
                    vt = dpool.tile([128, 256], BF16, tag=f"vT{j}")
                    nc.vector.tensor_add(vt[:], ps[:], vb_sb[:])
                    vT_sb.append(vt)

                # ---------------- attention ----------------
                out_sb = [dpool.tile([128, S], FP32, tag=f"o{ci}", name=f"o{ci}")
                          for ci in range(2)]
                for half in range(2):
                    qlo = 512 * half
                    av_ps = [pspool.tile([128, 512], FP32, tag="ps",
                                         name="avps") for _ in range(2)]
                    den_ps = [pspool.tile([128, 512], FP32, tag="ps",
                                          name="denps") for _ in range(2)]
                    for g in range(2):
                        # rows other than {0,32,64,96} are never written by
                        # the den matmuls; zero them so the full-tile clamp +
                        # reciprocal below can't hit stale NaN/Inf garbage.
                        nc.vector.memset(den_ps[g][:], 0.0)
                    last_kj = 4 * half + 3
                    for kj in range(last_kj + 1):
                        d = max(0, 128 * kj - qlo)  # cols left of diagonal
                        for g in range(2):
                            exs = []
                            for t in range(2):
                                # 2 heads per (128,1024) scores tile: 2 PSUM
                                # banks, so two tiles double-buffer in 4 banks
                                # and TensorE overlaps ScalarE's exp.
                                scp = scpool.tile([128, 1024], FP32, tag="sc",
                                                  name="scp")
                                for j in range(2):
                                    i = 2 * t + j
                                    nc.tensor.matmul(
                                        scp[:, 512 * j + d:512 * j + 512],
                                        lhsT=k_sb[g][32 * i:32 * i + 32,
                                                     128 * kj:128 * kj + 128],
                                        rhs=q_sb[g][32 * i:32 * i + 32,
                                                    qlo + d:qlo + 512],
                                        start=True, stop=True,
                                        tile_position=(32 * i, 0))
                                ex = epool.tile([128, 1024], BF16, tag="exp",
                                                name="ex")
                                sc3 = scp[:].rearrange("p (h c) -> p h c", h=2)
                                ex3 = ex[:].rearrange("p (h c) -> p h c", h=2)
                                nc.scalar.activation(ex3[:, :, d:512],
                                                     sc3[:, :, d:512], EXP)
                                if kj >= 4 * half:
                                    # strict causal mask on the diagonal block
                                    msl = ex3[:, :, d:d + 128]
                                    nc.vector.tensor_mul(
                                        msl, msl,
                                        maskT_sb[:, None, :].broadcast_to(
                                            (128, 2, 128)))
                                exs.append(ex)
                            for i in range(4):
                                h = 4 * g + i
                                ex = exs[i // 2]
                                exoff = 512 * (i % 2)
                                nc.tensor.matmul(
                                    av_ps[g][32 * i:32 * i + 32, d:512],
                                    lhsT=vT_sb[kj][:, 32 * h:32 * h + 32],
                                    rhs=ex[:, exoff + d:exoff + 512],
                                    start=(kj == 0), stop=(kj == last_kj),
                                    skip_group_check=True,
                                    tile_position=(0, 32 * i))
                                nc.tensor.matmul(
                                    den_ps[g][32 * i:32 * i + 1, d:512],
                                    lhsT=ones_col[:],
                                    rhs=ex[:, exoff + d:exoff + 512],
                                    start=(kj == 0), stop=(kj == last_kj),
                                    skip_group_check=True,
                                    tile_position=(0, 32 * i))

                    # ------- softmax division + output assembly -------
                    # den rows live at partitions {0,32,64,96}; evacuate the
                    # whole tile (clamped so 1/x is finite everywhere) and let
                    # a select-matrix matmul broadcast row 32i over rows
                    # 32i..32i+31.
                    for g in range(2):
                        den_all = spool.tile([128, 512], FP32, tag="den",
                                             name=f"den{g}")
                        nc.vector.tensor_scalar_max(den_all[:], den_ps[g][:],
                                                    1e-30)
                        recip_f = spool.tile([128, 512], FP32, tag="recf",
                                             name=f"recf{g}")
                        nc.vector.reciprocal_approx_fast(recip_f[:], den_all[:])
                        bc = pspool.tile([128, 512], FP32, tag="ps")
                        nc.tensor.matmul(bc[:], lhsT=sel_sb[:],
                                         rhs=recip_f[:], start=True, stop=True)
                        bc_sb = spool.tile([128, 512], FP32, tag=f"bcs{g}")
                        nc.vector.tensor_copy(bc_sb[:], bc[:])
                        nc.vector.tensor_mul(out_sb[g][:, qlo:qlo + 512],
                                             av_ps[g][:], bc_sb[:])

                for ci in range(2):
                    nc.sync.dma_start(out=out[b, 128 * ci:128 * ci + 128, :],
                                      in_=out_sb[ci][:])

    nc.compile()
    return nc


def _host_consts(q_w, q_b, kv_w, kv_b):
    scale = 1.0 / np.sqrt(DK)
    qk_wT = np.concatenate([q_w.T * scale, kv_w[:256].T], axis=1)
    qk_b_ = np.stack([q_b[:128] * scale, q_b[128:] * scale,
                      kv_b[:128], kv_b[128:256]], axis=1).astype(np.float32)
    v_wT = kv_w[256:].T
    v_b_ = np.broadcast_to(kv_b[256:][None, :], (128, 256)).copy()
    kidx = np.arange(128)
    maskT = (kidx[:, None] < kidx[None, :]).astype(np.float32)
    sel_m = (kidx[:, None] == 32 * (kidx[None, :] // 32)).astype(np.float32)
    return {
        "qk_wT": qk_wT.astype(BF16_NP),
        "qk_b": qk_b_,
        "v_wT": v_wT.astype(BF16_NP),
        "v_b": v_b_.astype(BF16_NP),
        "maskT": maskT.astype(BF16_NP),
        "sel": sel_m,
    }


def get_program():
    if "nc" not in _cache:
        _cache["nc"] = _build()
    return _cache["nc"]


def run(x, q_w, q_b, kv_w, kv_b, trace=False):
    nc = get_program()
    consts = _host_consts(np.asarray(q_w, np.float32), np.asarray(q_b, np.float32),
                          np.asarray(kv_w, np.float32), np.asarray(kv_b, np.float32))
    x = np.asarray(x, np.float32)
    in_maps = []
    for ci in range(N_CORES):
        m = dict(consts)
        m["xs"] = np.ascontiguousarray(
            x[ci * BPC:(ci + 1) * BPC].reshape(BPC, C, S))
        in_maps.append(m)
    res = run_bass_kernel_spmd(nc, in_maps, list(range(N_CORES)), trace=trace)
    outs = [res.results[ci]["out"] for ci in range(N_CORES)]
    full = np.concatenate(outs, axis=0).reshape(B, C, H, W).astype(np.float32)
    return full, res


def kernel(x, q_w, q_b, kv_w, kv_b):
    full, _ = run(x, q_w, q_b, kv_w, kv_b, trace=False)
    return full
